# revision 25
# baseline (speedup 1.0000x reference)
"""Trainium2 Bass kernel for the DrugEncoder GNN (2x GCNConv + GraphNorm + pool).

Self-contained: host-side index preprocessing + two SPMD Bass launches on 8
NeuronCores.

Math restructuring (vs the naive reference graph):
- GCN layer 1 aggregates in the 64-dim input space BEFORE the W1 matmul
  (aggregation and the linear map commute), halving gather traffic.
- GCN layer 2 + global mean pool collapse into `(P @ h2) @ W2 + b2` where
  P[g, r] = (1/n_g) * sum_{edges r->c, c in g} dis_c dis_r  (+ self loops)
  is index-only data built on the host. This removes the second edge gather
  entirely.
- GraphNorm1 output is pre-scaled by dis_r on the device ("y"); layer-1
  messages are a pure gather + segment-sum; the target-side dis_c factor is
  applied per 128-column block AFTER aggregation (it is constant per column).
- Self-loops are not gathered: launch 1 also emits yT = dis * h0^T and the
  self term dis_c^2 h0_c is added per block on the device.

Sharding: graphs are slotted (256-node slots, 32 graphs per core) so that all
per-graph and per-block structure is static and identical across the 8 cores
(SPMD); per-core variability lives in data streams only.

Launch 1: per-core GraphNorm1 -> y shard (node-major bf16) + yT (feat-major
fp32). Host reassembles the full slotted y into the lo/hi gather tables.
Launch 2: edge gather (dma_gather on lo/hi half tables, int16 indices),
segment-sum via PE matmuls with fp8 0/1 indicator tiles streamed from HBM,
dis_c + self-loop fixup per block, W1+ReLU, then per-quad (4 blocks = 2
graphs) GraphNorm2 + pool matmuls pipelined into the gather phase. Host sums
the 8 partial pools and adds b2.
"""
import os
import sys

sys.path.insert(0, "/opt/trn_rl_repo")

import numpy as np

import concourse.bacc as bacc
import concourse.bass as bass
import concourse.mybir as mybir
import concourse.tile as tile
from concourse import library_config
from concourse.bass_utils import run_bass_kernel_spmd

F32 = mybir.dt.float32
BF16 = mybir.dt.bfloat16
FP8 = mybir.dt.float8e4
I16 = mybir.dt.int16
AF = mybir.ActivationFunctionType
OP = mybir.AluOpType
AX = mybir.AxisListType

C = 8            # cores
G = 256          # graphs
SLOT = 256       # nodes per graph slot
GPC = G // C     # graphs per core
NPC = GPC * SLOT  # slotted nodes per core (8192)
NBLK = NPC // 128  # node blocks per core (64)
BPB = 8          # blocks per batch
NBATCH = NBLK // BPB
HALF = 32768     # lo/hi split of slotted global rows (C*NPC = 65536)
D0, DH, DO = 64, 128, 64
EPS = 1e-5

LAST_EXEC_NS = []  # filled per launch when BASS_TRACE is set


# --------------------------------------------------------------------------
# Host-side preprocessing (index data only)
# --------------------------------------------------------------------------

def _slot_nodes(batch):
    """slotted id = gperm[g]*SLOT + pos; gperm balances node counts per core."""
    counts = np.bincount(batch, minlength=G).astype(np.int64)
    assert counts.max() <= SLOT, f"graph size {counts.max()} > SLOT {SLOT}"
    gperm = _graph_perm(counts)
    starts = np.zeros(G + 1, np.int64)
    np.cumsum(counts, out=starts[1:])
    pos = np.arange(len(batch)) - starts[batch]
    slotted = gperm[batch] * SLOT + pos
    return slotted.astype(np.int64), counts, gperm


def _graph_perm(counts):
    """Assign graphs to cores balancing node counts (greedy, largest first).
    Returns perm[g] = slot index (core*GPC + slot_in_core)."""
    order = np.argsort(-counts, kind="stable")
    loads = np.zeros(C, np.int64)
    fill = np.zeros(C, np.int64)
    perm = np.zeros(G, np.int64)
    for g in order:
        k = int(np.argmin(loads + np.where(fill >= GPC, 1 << 40, 0)))
        perm[g] = k * GPC + fill[k]
        fill[k] += 1
        loads[k] += counts[g]
    return perm


def _preprocess(edge_index, batch):
    N = batch.shape[0]
    row = np.asarray(edge_index[0], dtype=np.int64)
    col = np.asarray(edge_index[1], dtype=np.int64)
    batch = np.asarray(batch, dtype=np.int64)
    slotted, counts, gperm = _slot_nodes(batch)

    deg = np.bincount(col, minlength=N).astype(np.float64) + 1.0
    dis = (1.0 / np.sqrt(deg)).astype(np.float32)

    srow = slotted[row]
    scol = slotted[col]
    sdis = np.zeros(C * NPC, np.float32)
    sdis[slotted] = dis

    # Edges only (no self loops - handled via the yT side input).
    per_core = []
    for k in range(C):
        lo_n, hi_n = k * NPC, (k + 1) * NPC
        m = (scol >= lo_n) & (scol < hi_n)
        r = srow[m]
        lc = scol[m] - lo_n
        half = (r >= HALF).astype(np.int64)
        per_core.append((r, lc, half))

    cnt = np.zeros((C, NBLK, 2), np.int64)
    for k in range(C):
        r, lc, half = per_core[k]
        np.add.at(cnt[k], (lc // 128, half), 1)
    S = cnt.max(axis=0)  # SPMD: identical slot structure across cores

    for bi in range(NBATCH):
        for h in (0, 1):
            tot = int(S[bi * BPB:(bi + 1) * BPB, h].sum())
            S[(bi + 1) * BPB - 1, h] += (-tot) % 128

    off = np.zeros((NBLK, 2), np.int64)
    run_info = []
    cur = 0
    for bi in range(NBATCH):
        blks = list(range(bi * BPB, (bi + 1) * BPB))
        lo_start = cur
        for b in blks:  # lo run: all blocks' lo slots, contiguous
            off[b, 0] = cur
            cur += S[b, 0]
        hi_start = cur
        for b in blks:
            off[b, 1] = cur
            cur += S[b, 1]
        run_info.append((lo_start, hi_start - lo_start, hi_start, cur - hi_start))
    total_slots = cur
    T_total = total_slots // 128

    # units in QUAD order: per batch, quads of 4 blocks complete (lo+hi)
    # before the next quad opens, bounding concurrent PSUM accum groups.
    units = []  # (tile, block, start, stop)
    first = set()
    for bi in range(NBATCH):
        for q in range(BPB // 4):
            for h in (0, 1):
                for b in range(bi * BPB + q * 4, bi * BPB + (q + 1) * 4):
                    s0, s1 = int(off[b, h]), int(off[b, h] + S[b, h])
                    for t in range(s0 // 128, (s1 - 1) // 128 + 1):
                        units.append([t, b, b not in first, False])
                        first.add(b)
    last_of_block = {}
    for j, (t, b, st, sp) in enumerate(units):
        last_of_block[b] = j
    for b, j in last_of_block.items():
        units[j][3] = True
    U = len(units)

    unit_of = {}
    for j, (t, b, st, sp) in enumerate(units):
        unit_of[(t, b)] = j

    idx16 = np.zeros((C, total_slots), np.int16)
    seg8 = np.zeros((C, 128, U, 128), np.uint8)  # bitcast to fp8 later
    for k in range(C):
        r, lc, half = per_core[k]
        blk = lc // 128
        bi_e = blk // BPB
        # group by (batch, half, block); sort by source row within a group
        # for DRAM locality in the gather.
        order = np.lexsort((r, blk, bi_e * 2 + half))
        r, lc, half, blk = r[order], lc[order], half[order], blk[order]
        grp = blk * 2 + half
        change = np.flatnonzero(np.diff(grp, prepend=-1))
        lens = np.diff(np.append(change, len(grp)))
        idx_in_grp = np.arange(len(grp)) - np.repeat(change, lens)
        slot = off[blk, half] + idx_in_grp
        idx16[k, slot] = (r - half * HALF).astype(np.int16)
        tile_ = slot // 128
        p = slot % 128
        uj = np.fromiter((unit_of[(t, b)] for t, b in zip(tile_, blk)),
                         dtype=np.int64, count=len(tile_))
        seg8[k, p, uj, (lc - blk * 128)] = 0x38  # fp8e4m3 bit pattern of 1.0

    return dict(
        slotted=slotted, counts=counts, gperm=gperm, dis=dis, sdis=sdis,
        S=S, off=off, run_info=run_info, units=units, U=U,
        T_total=T_total, total_slots=total_slots,
        idx16=idx16, seg8=seg8, batch=batch,
        row=row, col=col,
    )


def _build_P(pp):
    row, col, batch = pp["row"], pp["col"], pp["batch"]
    dis, counts, slotted = pp["dis"], pp["counts"], pp["slotted"]
    g_of_col = batch[col]
    w = dis[col].astype(np.float64) * dis[row].astype(np.float64)
    flat = g_of_col * (C * NPC) + slotted[row]
    P = np.bincount(flat, weights=w, minlength=G * C * NPC)
    flat2 = batch * (C * NPC) + slotted
    P += np.bincount(flat2, weights=dis.astype(np.float64) ** 2,
                     minlength=G * C * NPC)
    P = P.reshape(G, C * NPC)
    P /= np.maximum(counts[:, None], 1).astype(np.float64)
    return P.astype(np.float32)


def _wrap_idx16(idx):
    """[total] int16 -> [128, total//16] wrapped (j -> [j%16, j//16], x8)."""
    lay = idx.reshape(-1, 16).T  # [16, total/16]
    return np.tile(lay, (8, 1)).copy()


# --------------------------------------------------------------------------
# Launch 1: GraphNorm1 -> y shard (node-major bf16) + yT (feat-major fp32)
# --------------------------------------------------------------------------

def _build_launch1():
    nc = bacc.Bacc("TRN2", target_bir_lowering=False, debug=False)
    xT = nc.dram_tensor("xT", [D0, NPC], BF16, kind="ExternalInput")
    dis_in = nc.dram_tensor("dis_sb", [128, NBLK], F32, kind="ExternalInput")
    disfree = nc.dram_tensor("disfree", [D0, NPC], F32, kind="ExternalInput")
    invn = nc.dram_tensor("invn", [D0, GPC], F32, kind="ExternalInput")
    msv = nc.dram_tensor("msv", [D0, 1], F32, kind="ExternalInput")
    wv = nc.dram_tensor("wv", [D0, 1], F32, kind="ExternalInput")
    bv = nc.dram_tensor("bv", [D0, 1], F32, kind="ExternalInput")
    ident = nc.dram_tensor("ident", [128, 128], F32, kind="ExternalInput")
    epsv = nc.dram_tensor("epsv", [D0, 1], F32, kind="ExternalInput")
    y_out = nc.dram_tensor("y_out", [NPC, D0], BF16, kind="ExternalOutput")
    yT_out = nc.dram_tensor("yT_out", [D0, NPC], F32, kind="ExternalOutput")

    with tile.TileContext(nc) as tc:
        with tc.tile_pool(name="sb", bufs=1) as sb, \
             tc.tile_pool(name="ps", bufs=4, space="PSUM") as ps:
            xT_sb = sb.tile([D0, NPC], BF16)
            nc.sync.dma_start(out=xT_sb[:], in_=xT[:])
            dis_sb = sb.tile([128, NBLK], F32)
            nc.sync.dma_start(out=dis_sb[:], in_=dis_in[:])
            disfree_sb = sb.tile([D0, NPC], F32)
            nc.sync.dma_start(out=disfree_sb[:], in_=disfree[:])
            invn_sb = sb.tile([D0, GPC], F32)
            nc.sync.dma_start(out=invn_sb[:], in_=invn[:])
            ms_sb = sb.tile([D0, 1], F32)
            nc.sync.dma_start(out=ms_sb[:], in_=msv[:])
            w_sb = sb.tile([D0, 1], F32)
            nc.sync.dma_start(out=w_sb[:], in_=wv[:])
            b_sb = sb.tile([D0, 1], F32)
            nc.sync.dma_start(out=b_sb[:], in_=bv[:])
            id_sb = sb.tile([128, 128], F32)
            nc.sync.dma_start(out=id_sb[:], in_=ident[:])
            eps_sb = sb.tile([D0, 1], F32)
            nc.sync.dma_start(out=eps_sb[:], in_=epsv[:])

            sums = sb.tile([D0, GPC], F32)
            sumsq = sb.tile([D0, GPC], F32)
            nc.vector.reduce_sum(
                out=sums[:], in_=xT_sb[:].rearrange("p (g s) -> p g s", s=SLOT),
                axis=AX.X)
            sqf = sb.tile([D0, NPC], BF16)
            nc.vector.tensor_tensor(out=sqf[:], in0=xT_sb[:], in1=xT_sb[:],
                                    op=OP.mult)
            nc.vector.reduce_sum(
                out=sumsq[:], in_=sqf[:].rearrange("p (g s) -> p g s", s=SLOT),
                axis=AX.X)

            mu = sb.tile([D0, GPC], F32)
            nc.vector.tensor_tensor(out=mu[:], in0=sums[:], in1=invn_sb[:], op=OP.mult)
            m2 = sb.tile([D0, GPC], F32)
            nc.vector.tensor_scalar(out=m2[:], in0=mu[:], scalar1=ms_sb[:, :1],
                                    scalar2=None, op0=OP.mult)
            ex2 = sb.tile([D0, GPC], F32)
            nc.vector.tensor_tensor(out=ex2[:], in0=sumsq[:], in1=invn_sb[:], op=OP.mult)
            var = sb.tile([D0, GPC], F32)
            nc.vector.tensor_tensor(out=var[:], in0=m2[:], in1=mu[:], op=OP.mult)
            nc.vector.tensor_scalar(out=var[:], in0=var[:], scalar1=-2.0,
                                    scalar2=None, op0=OP.mult)
            nc.vector.tensor_tensor(out=var[:], in0=var[:], in1=ex2[:], op=OP.add)
            m2sq = sb.tile([D0, GPC], F32)
            nc.vector.tensor_tensor(out=m2sq[:], in0=m2[:], in1=m2[:], op=OP.mult)
            nc.vector.tensor_tensor(out=var[:], in0=var[:], in1=m2sq[:], op=OP.add)
            std = sb.tile([D0, GPC], F32)
            nc.scalar.activation(out=std[:], in_=var[:], func=AF.Sqrt,
                                 bias=eps_sb[:, :1])
            inv = sb.tile([D0, GPC], F32)
            nc.vector.reciprocal(out=inv[:], in_=std[:])
            Av = sb.tile([D0, GPC], F32)
            nc.vector.tensor_scalar(out=Av[:], in0=inv[:], scalar1=w_sb[:, :1],
                                    scalar2=None, op0=OP.mult)
            Bv = sb.tile([D0, GPC], F32)
            nc.vector.tensor_tensor(out=Bv[:], in0=Av[:], in1=m2[:], op=OP.mult)
            nc.vector.tensor_scalar(out=Bv[:], in0=Bv[:], scalar1=-1.0,
                                    scalar2=b_sb[:, :1], op0=OP.mult, op1=OP.add)

            h0T = sb.tile([D0, NPC], F32)
            for gs in range(GPC):
                nc.scalar.activation(
                    out=h0T[:, gs * SLOT:(gs + 1) * SLOT],
                    in_=xT_sb[:, gs * SLOT:(gs + 1) * SLOT],
                    func=AF.Identity, scale=Av[:, gs:gs + 1],
                    bias=Bv[:, gs:gs + 1])

            # yT = dis * h0T  (fp32, feat-major; zero on empty slots)
            yT_sb = sb.tile([D0, NPC], F32)
            nc.vector.tensor_tensor(out=yT_sb[:], in0=h0T[:], in1=disfree_sb[:],
                                    op=OP.mult)
            nc.sync.dma_start(out=yT_out[:], in_=yT_sb[:])

            y_sb = sb.tile([128, NBLK, D0], BF16)
            for cki in range(NBLK):
                tr = ps.tile([128, D0], F32, tag="tr")
                nc.tensor.transpose(out=tr[:], in_=h0T[:, cki * 128:(cki + 1) * 128],
                                    identity=id_sb[:D0, :D0])
                nc.scalar.activation(out=y_sb[:, cki, :], in_=tr[:],
                                     func=AF.Copy,
                                     scale=dis_sb[:, cki:cki + 1])
            nc.sync.dma_start(
                out=y_out.rearrange("(c p) f -> p c f", p=128), in_=y_sb[:])
    nc.compile()
    return nc


# --------------------------------------------------------------------------
# Launch 2: gather + segment-sum + W1/ReLU + pipelined GraphNorm2 + pooling
# --------------------------------------------------------------------------

def _build_launch2(pp):
    total_slots, U = pp["total_slots"], pp["U"]
    units, run_info = pp["units"], pp["run_info"]
    T_batch = [(ll + hl) // 128 for (_, ll, _, hl) in run_info]
    T_batch_max = max(T_batch)
    # units per batch (contiguous in unit order)
    ub0 = [0] * (NBATCH + 1)
    for j, (t, b, st, sp) in enumerate(units):
        ub0[b // BPB + 1] = j + 1
    U_batch_max = max(ub0[i + 1] - ub0[i] for i in range(NBATCH))

    nc = bacc.Bacc("TRN2", target_bir_lowering=False, debug=False,
                   num_swdge_queues=4)
    y_lo = nc.dram_tensor("y_lo", [HALF, 128], BF16, kind="ExternalInput")
    y_hi = nc.dram_tensor("y_hi", [HALF, 128], BF16, kind="ExternalInput")
    idxs = nc.dram_tensor("idxs", [128, total_slots // 16], I16, kind="ExternalInput")
    seg8 = nc.dram_tensor("seg8", [128, U * 128], FP8, kind="ExternalInput")
    disblk = nc.dram_tensor("disblk", [D0, NPC], BF16, kind="ExternalInput")
    yT_in = nc.dram_tensor("yT_in", [D0, NPC], F32, kind="ExternalInput")
    ident = nc.dram_tensor("ident", [128, 128], F32, kind="ExternalInput")
    PT = nc.dram_tensor("PT", [NPC, G], BF16, kind="ExternalInput")
    W1 = nc.dram_tensor("W1", [D0, DH], BF16, kind="ExternalInput")
    b1 = nc.dram_tensor("b1", [DH, 1], F32, kind="ExternalInput")
    W2 = nc.dram_tensor("W2", [DH, DO], F32, kind="ExternalInput")
    gn2w = nc.dram_tensor("gn2w", [DH, 1], F32, kind="ExternalInput")
    gn2b = nc.dram_tensor("gn2b", [DH, 1], F32, kind="ExternalInput")
    gn2ms = nc.dram_tensor("gn2ms", [DH, 1], F32, kind="ExternalInput")
    invn2 = nc.dram_tensor("invn2", [DH, GPC], F32, kind="ExternalInput")
    npad = nc.dram_tensor("npad", [DH, GPC], F32, kind="ExternalInput")
    epsv = nc.dram_tensor("epsv", [DH, 1], F32, kind="ExternalInput")
    part = nc.dram_tensor("part", [G, DO], F32, kind="ExternalOutput")

    nc.gpsimd.load_library(library_config.mlp)
    with tile.TileContext(nc) as tc:
        with tc.tile_pool(name="cst", bufs=1) as cst:
            idxs_sb = cst.tile([128, total_slots // 16], I16)
            id_sb = cst.tile([128, 128], F32)
            nc.sync.dma_start(out=id_sb[:], in_=ident[:])
            W1_sb = cst.tile([D0, DH], BF16)
            nc.sync.dma_start(out=W1_sb[:], in_=W1[:])
            b1_sb = cst.tile([DH, 1], F32)
            nc.sync.dma_start(out=b1_sb[:], in_=b1[:])
            W2_sb = cst.tile([DH, DO], F32)
            nc.sync.dma_start(out=W2_sb[:], in_=W2[:])
            gn2w_sb = cst.tile([DH, 1], F32)
            nc.sync.dma_start(out=gn2w_sb[:], in_=gn2w[:])
            gn2b_sb = cst.tile([DH, 1], F32)
            nc.sync.dma_start(out=gn2b_sb[:], in_=gn2b[:])
            gn2ms_sb = cst.tile([DH, 1], F32)
            nc.sync.dma_start(out=gn2ms_sb[:], in_=gn2ms[:])
            invn2_sb = cst.tile([DH, GPC], F32)
            nc.sync.dma_start(out=invn2_sb[:], in_=invn2[:])
            npad_sb = cst.tile([DH, GPC], F32)
            nc.sync.dma_start(out=npad_sb[:], in_=npad[:])
            eps_sb = cst.tile([DH, 1], F32)
            nc.sync.dma_start(out=eps_sb[:], in_=epsv[:])

            relu_b1 = cst.tile([DH, 1], F32)
            nc.scalar.activation(out=relu_b1[:], in_=b1_sb[:], func=AF.Relu)
            relu_b1sq = cst.tile([DH, 1], F32)
            nc.vector.tensor_tensor(out=relu_b1sq[:], in0=relu_b1[:],
                                    in1=relu_b1[:], op=OP.mult)

            h1T = cst.tile([DH, NPC], F32)

            GATHER_CHUNK = int(os.environ.get("GATHER_CHUNK", "2048"))

            NPB = NPC // NBATCH  # node columns per batch (1024)
            with tc.tile_pool(name="msg", bufs=24) as msgp, \
                 tc.tile_pool(name="segs", bufs=2) as segsp, \
                 tc.tile_pool(name="ytb", bufs=2) as ytbp, \
                 tc.tile_pool(name="disb", bufs=2) as disbp, \
                 tc.tile_pool(name="aggsb", bufs=4) as aggsbp, \
                 tc.tile_pool(name="aggps", bufs=4, space="PSUM") as aggpsp, \
                 tc.tile_pool(name="h1ps", bufs=1, space="PSUM") as h1psp, \
                 tc.tile_pool(name="gn", bufs=6) as gnp, \
                 tc.tile_pool(name="pe", bufs=4) as pep, \
                 tc.tile_pool(name="peps", bufs=1, space="PSUM") as pepsp, \
                 tc.tile_pool(name="poolps", bufs=2, space="PSUM") as poolpsp:
                pool0 = poolpsp.tile([128, DH], F32, tag="pool", name="pool0")
                pool1 = poolpsp.tile([128, DH], F32, tag="pool", name="pool1")

                tile_map = {}  # global slot-tile index -> (chunk_tile, local_t)
                seg_tiles = []
                ytb_tiles = []
                disb_tiles = []
                agg_tiles = {}
                call_idx = [0]

                CT = GATHER_CHUNK // 128  # slot-tiles per chunk tile

                def emit_gather(src, slot0, nslots):
                    # one msgs tile per chunk -> units fire chunk-by-chunk
                    done = 0
                    while done < nslots:
                        chunk = min(GATHER_CHUNK, nslots - done)
                        s0 = slot0 + done
                        mt = msgp.tile([128, CT, 128], BF16, tag="msgs")
                        nc.gpsimd.dma_gather(
                            mt[:, :chunk // 128, :],
                            src[:],
                            idxs_sb[:, s0 // 16:(s0 + chunk) // 16],
                            chunk, chunk, 128, single_packet=False,
                            queue_num=call_idx[0] % 4)
                        for j in range(chunk // 128):
                            tile_map[s0 // 128 + j] = (mt, j)
                        call_idx[0] += 1
                        done += chunk

                for bi in range(NBATCH):
                    lo_s, lo_l, hi_s, hi_l = run_info[bi]
                    nc.sync.dma_start(
                        out=idxs_sb[:, lo_s // 16:(hi_s + hi_l) // 16],
                        in_=idxs[:, lo_s // 16:(hi_s + hi_l) // 16])
                    emit_gather(y_lo, lo_s, lo_l)
                    emit_gather(y_hi, hi_s, hi_l)
                    nu = ub0[bi + 1] - ub0[bi]
                    segt = segsp.tile([128, U_batch_max * 128], FP8, tag="seg")
                    nc.sync.dma_start(
                        out=segt[:, :nu * 128],
                        in_=seg8[:, ub0[bi] * 128:ub0[bi + 1] * 128])
                    seg_tiles.append(segt)
                    ytb = ytbp.tile([D0, NPB], F32, tag="ytb")
                    nc.sync.dma_start(out=ytb[:],
                                      in_=yT_in[:, bi * NPB:(bi + 1) * NPB])
                    ytb_tiles.append(ytb)
                    disb = disbp.tile([D0, NPB], BF16, tag="disb")
                    nc.sync.dma_start(out=disb[:],
                                      in_=disblk[:, bi * NPB:(bi + 1) * NPB])
                    disb_tiles.append(disb)
                    for pos in range(BPB):
                        agg_tiles[(bi, pos)] = aggpsp.tile(
                            [D0, 128], F32, tag="agg", name=f"agg{bi}_{pos}")

                blk_sums = {}
                blk_sumsq = {}

                def finish_block(b):
                    bi = b // BPB
                    agg = agg_tiles[(bi, b % BPB)]
                    # agg_sb = (agg + yT_blk) * dis_blk  -- self loop + dis_c
                    lb = (b % BPB) * 128
                    tmp = aggsbp.tile([D0, 128], BF16, tag="tmp")
                    nc.vector.tensor_tensor(
                        out=tmp[:], in0=agg[:],
                        in1=ytb_tiles[bi][:, lb:lb + 128], op=OP.add)
                    agg_sb = aggsbp.tile([D0, 128], BF16, tag="aggsb")
                    nc.vector.tensor_tensor(
                        out=agg_sb[:], in0=tmp[:],
                        in1=disb_tiles[bi][:, lb:lb + 128], op=OP.mult)
                    h1ps = h1psp.tile([DH, 128], F32, tag="h1ps")
                    nc.tensor.matmul(out=h1ps[:], lhsT=W1_sb[:], rhs=agg_sb[:],
                                     start=True, stop=True)
                    nc.scalar.activation(
                        out=h1T[:, b * 128:(b + 1) * 128], in_=h1ps[:],
                        func=AF.Relu, bias=b1_sb[:, :1])
                    bs = gnp.tile([DH, 1], F32, tag="bsum")
                    nc.vector.reduce_sum(
                        out=bs[:], in_=h1T[:, b * 128:(b + 1) * 128], axis=AX.X)
                    blk_sums[b] = bs
                    sqb = gnp.tile([DH, 128], F32, tag="sqb")
                    nc.vector.tensor_tensor(
                        out=sqb[:], in0=h1T[:, b * 128:(b + 1) * 128],
                        in1=h1T[:, b * 128:(b + 1) * 128], op=OP.mult)
                    bq = gnp.tile([DH, 1], F32, tag="bsumsq")
                    nc.vector.reduce_sum(out=bq[:], in_=sqb[:], axis=AX.X)
                    blk_sumsq[b] = bq

                def finish_quad(qi):
                    # blocks 4qi..4qi+3 are done -> graphs 2qi, 2qi+1
                    g0 = 2 * qi
                    n0 = g0 * SLOT
                    sums = gnp.tile([DH, 2], F32, tag="sums")
                    sumsq = gnp.tile([DH, 2], F32, tag="sumsq")
                    for gs in (0, 1):
                        b0 = 4 * qi + 2 * gs
                        nc.vector.tensor_tensor(
                            out=sums[:, gs:gs + 1], in0=blk_sums[b0][:],
                            in1=blk_sums[b0 + 1][:], op=OP.add)
                        nc.vector.tensor_tensor(
                            out=sumsq[:, gs:gs + 1], in0=blk_sumsq[b0][:],
                            in1=blk_sumsq[b0 + 1][:], op=OP.add)
                    # pad corrections (tensor_tensor only: tensor_scalar /
                    # copy / cast enter DVE 2-port mode and hard-block
                    # against SWDGE gather descriptor generation)
                    corr = gnp.tile([DH, 2], F32, tag="corr")
                    nc.vector.tensor_tensor(
                        out=corr[:], in0=npad_sb[:, g0:g0 + 2],
                        in1=relu_b1[:, 0:1].to_broadcast([DH, 2]), op=OP.mult)
                    nc.vector.tensor_tensor(out=sums[:], in0=sums[:], in1=corr[:],
                                            op=OP.subtract)
                    corr2 = gnp.tile([DH, 2], F32, tag="corr2")
                    nc.vector.tensor_tensor(
                        out=corr2[:], in0=npad_sb[:, g0:g0 + 2],
                        in1=relu_b1sq[:, 0:1].to_broadcast([DH, 2]), op=OP.mult)
                    nc.vector.tensor_tensor(out=sumsq[:], in0=sumsq[:],
                                            in1=corr2[:], op=OP.subtract)

                    mu = gnp.tile([DH, 2], F32, tag="mu")
                    nc.vector.tensor_tensor(out=mu[:], in0=sums[:],
                                            in1=invn2_sb[:, g0:g0 + 2], op=OP.mult)
                    m2 = gnp.tile([DH, 2], F32, tag="m2")
                    nc.vector.tensor_tensor(
                        out=m2[:], in0=mu[:],
                        in1=gn2ms_sb[:, 0:1].to_broadcast([DH, 2]), op=OP.mult)
                    ex2 = gnp.tile([DH, 2], F32, tag="ex2")
                    nc.vector.tensor_tensor(out=ex2[:], in0=sumsq[:],
                                            in1=invn2_sb[:, g0:g0 + 2], op=OP.mult)
                    mm = gnp.tile([DH, 2], F32, tag="mm")
                    nc.vector.tensor_tensor(out=mm[:], in0=m2[:], in1=mu[:],
                                            op=OP.mult)
                    var = gnp.tile([DH, 2], F32, tag="var")
                    nc.vector.tensor_tensor(out=var[:], in0=ex2[:], in1=mm[:],
                                            op=OP.subtract)
                    nc.vector.tensor_tensor(out=var[:], in0=var[:], in1=mm[:],
                                            op=OP.subtract)
                    m2sq = gnp.tile([DH, 2], F32, tag="m2sq")
                    nc.vector.tensor_tensor(out=m2sq[:], in0=m2[:], in1=m2[:],
                                            op=OP.mult)
                    nc.vector.tensor_tensor(out=var[:], in0=var[:], in1=m2sq[:],
                                            op=OP.add)
                    std = gnp.tile([DH, 2], F32, tag="std")
                    nc.scalar.activation(out=std[:], in_=var[:], func=AF.Sqrt,
                                         bias=eps_sb[:, :1])
                    inv = gnp.tile([DH, 2], F32, tag="inv")
                    nc.vector.reciprocal(out=inv[:], in_=std[:])
                    Av = gnp.tile([DH, 2], F32, tag="Av")
                    nc.vector.tensor_tensor(
                        out=Av[:], in0=gn2w_sb[:, 0:1].to_broadcast([DH, 2]),
                        in1=inv[:], op=OP.mult)
                    mm2 = gnp.tile([DH, 2], F32, tag="mm2")
                    nc.vector.tensor_tensor(out=mm2[:], in0=Av[:], in1=m2[:],
                                            op=OP.mult)
                    Bv = gnp.tile([DH, 2], F32, tag="Bv")
                    nc.vector.tensor_tensor(
                        out=Bv[:], in0=gn2b_sb[:, 0:1].to_broadcast([DH, 2]),
                        in1=mm2[:], op=OP.subtract)
                    for gs in (0, 1):
                        nc.scalar.activation(
                            out=h1T[:, n0 + gs * SLOT:n0 + (gs + 1) * SLOT],
                            in_=h1T[:, n0 + gs * SLOT:n0 + (gs + 1) * SLOT],
                            func=AF.Identity, scale=Av[:, gs:gs + 1],
                            bias=Bv[:, gs:gs + 1])
                    # pool matmuls for the 4 blocks
                    for b in range(4 * qi, 4 * qi + 4):
                        trp = pepsp.tile([128, 128], F32, tag="trp")
                        nc.tensor.transpose(out=trp[:],
                                            in_=h1T[:, b * 128:(b + 1) * 128],
                                            identity=id_sb[:])
                        h2nm = pep.tile([128, 128], BF16, tag="h2nm")
                        nc.scalar.activation(out=h2nm[:], in_=trp[:],
                                             func=AF.Copy)
                        PT_sb = pep.tile([128, G], BF16, tag="pt")
                        nc.sync.dma_start(out=PT_sb[:],
                                          in_=PT[b * 128:(b + 1) * 128, :])
                        nc.tensor.matmul(out=pool0[:], lhsT=PT_sb[:, 0:128],
                                         rhs=h2nm[:], start=(b == 0),
                                         stop=(b == NBLK - 1))
                        nc.tensor.matmul(out=pool1[:], lhsT=PT_sb[:, 128:256],
                                         rhs=h2nm[:], start=(b == 0),
                                         stop=(b == NBLK - 1))

                done_blocks = set()
                for j, (t, b, st, sp) in enumerate(units):
                    bi = b // BPB
                    u_in = j - ub0[bi]
                    agg = agg_tiles[(bi, b % BPB)]
                    mt, lt = tile_map[t]
                    nc.tensor.matmul(
                        out=agg[:],
                        lhsT=mt[:, lt, 0:D0],
                        rhs=seg_tiles[bi][:, u_in * 128:(u_in + 1) * 128],
                        start=st, stop=sp)
                    if sp:
                        finish_block(b)
                        done_blocks.add(b)
                        qi = b // 4
                        if all((4 * qi + i) in done_blocks for i in range(4)):
                            finish_quad(qi)

                # ---- final: pooled -> @W2 -> part ----
                for hh, pool in enumerate((pool0, pool1)):
                    pp_sb = pep.tile([128, DH], F32, tag="ppsb")
                    nc.scalar.activation(out=pp_sb[:], in_=pool[:], func=AF.Copy)
                    trp2 = pepsp.tile([128, 128], F32, tag="trp")
                    nc.tensor.transpose(out=trp2[:], in_=pp_sb[:], identity=id_sb[:])
                    ppT = pep.tile([128, 128], F32, tag="ppT")
                    nc.scalar.activation(out=ppT[:], in_=trp2[:], func=AF.Copy)
                    ops_ = pepsp.tile([128, 128], F32, tag="trp")
                    nc.tensor.matmul(out=ops_[:, :DO], lhsT=ppT[:], rhs=W2_sb[:],
                                     start=True, stop=True)
                    out_sb = pep.tile([128, DO], F32, tag="outsb")
                    nc.scalar.activation(out=out_sb[:], in_=ops_[:, :DO],
                                         func=AF.Copy)
                    nc.sync.dma_start(out=part[hh * 128:(hh + 1) * 128, :],
                                      in_=out_sb[:])
    nc.compile()
    return nc


# --------------------------------------------------------------------------
# Entry point
# --------------------------------------------------------------------------

def kernel(**inputs):
    global LAST_EXEC_NS
    LAST_EXEC_NS = []
    import ml_dtypes
    x = np.asarray(inputs["x"], np.float32)
    edge_index = np.asarray(inputs["edge_index"])
    batch = np.asarray(inputs["batch"])
    gn1_w = np.asarray(inputs["gn1_w"], np.float32)
    gn1_b = np.asarray(inputs["gn1_b"], np.float32)
    gn1_ms = np.asarray(inputs["gn1_ms"], np.float32)
    W1 = np.asarray(inputs["W1"], np.float32)
    b1 = np.asarray(inputs["b1"], np.float32)
    gn2_w = np.asarray(inputs["gn2_w"], np.float32)
    gn2_b = np.asarray(inputs["gn2_b"], np.float32)
    gn2_ms = np.asarray(inputs["gn2_ms"], np.float32)
    W2 = np.asarray(inputs["W2"], np.float32)
    b2 = np.asarray(inputs["b2"], np.float32)

    pp = _preprocess(edge_index, batch)
    P = _build_P(pp)
    counts, slotted, sdis = pp["counts"], pp["slotted"], pp["sdis"]
    invperm = np.argsort(pp["gperm"])  # slot -> original graph
    slot_counts = counts[invperm]      # counts ordered by slot

    trace = bool(os.environ.get("BASS_TRACE"))

    # slotted x
    xs = np.zeros((C * NPC, D0), np.float32)
    xs[slotted] = x
    ident = np.eye(128, dtype=np.float32)

    # ---- launch 1 ----
    nc1 = _build_launch1()
    in_maps1 = []
    for k in range(C):
        xT_k = np.ascontiguousarray(
            xs[k * NPC:(k + 1) * NPC].T).astype(ml_dtypes.bfloat16)
        dis_k = np.ascontiguousarray(
            sdis[k * NPC:(k + 1) * NPC].reshape(NBLK, 128).T)
        disfree_k = np.broadcast_to(
            sdis[k * NPC:(k + 1) * NPC][None, :], (D0, NPC)).copy()
        n_k = slot_counts[k * GPC:(k + 1) * GPC].astype(np.float64)
        invn_k = np.broadcast_to(
            (1.0 / np.maximum(n_k, 1.0)).astype(np.float32)[None, :],
            (D0, GPC)).copy()
        in_maps1.append({
            "xT": xT_k, "dis_sb": dis_k, "disfree": disfree_k, "invn": invn_k,
            "msv": gn1_ms[:, None].copy(), "wv": gn1_w[:, None].copy(),
            "bv": gn1_b[:, None].copy(), "ident": ident,
            "epsv": np.full((D0, 1), EPS, np.float32),
        })
    res1 = run_bass_kernel_spmd(nc1, in_maps1, core_ids=list(range(C)),
                                trace=trace)
    if res1.exec_time_ns is not None:
        LAST_EXEC_NS.append(res1.exec_time_ns)
    y = np.concatenate([res1.results[k]["y_out"] for k in range(C)], axis=0)
    yTs = [res1.results[k]["yT_out"] for k in range(C)]
    yp = np.zeros((C * NPC, 128), ml_dtypes.bfloat16)
    yp[:, :D0] = y
    y_lo = np.ascontiguousarray(yp[:HALF])
    y_hi = np.ascontiguousarray(yp[HALF:])

    # ---- launch 2 ----
    nc2 = _build_launch2(pp)
    seg8_all = pp["seg8"].reshape(C, 128, pp["U"] * 128).view(
        ml_dtypes.float8_e4m3)
    in_maps2 = []
    for k in range(C):
        n_k = slot_counts[k * GPC:(k + 1) * GPC].astype(np.float64)
        invn2_k = np.broadcast_to(
            (1.0 / np.maximum(n_k, 1.0)).astype(np.float32)[None, :],
            (DH, GPC)).copy()
        npad_k = np.broadcast_to(
            (SLOT - n_k).astype(np.float32)[None, :], (DH, GPC)).copy()
        PT_k = np.ascontiguousarray(
            P[:, k * NPC:(k + 1) * NPC].T.astype(ml_dtypes.bfloat16))
        disblk_k = np.broadcast_to(
            sdis[k * NPC:(k + 1) * NPC].astype(ml_dtypes.bfloat16)[None, :],
            (D0, NPC)).copy()
        in_maps2.append({
            "y_lo": y_lo, "y_hi": y_hi,
            "idxs": _wrap_idx16(pp["idx16"][k]),
            "seg8": np.ascontiguousarray(seg8_all[k]),
            "disblk": disblk_k, "yT_in": np.asarray(yTs[k], np.float32),
            "ident": ident, "PT": PT_k,
            "W1": W1.astype(ml_dtypes.bfloat16), "b1": b1[:, None].copy(),
            "W2": W2,
            "gn2w": gn2_w[:, None].copy(), "gn2b": gn2_b[:, None].copy(),
            "gn2ms": gn2_ms[:, None].copy(),
            "invn2": invn2_k, "npad": npad_k,
            "epsv": np.full((DH, 1), EPS, np.float32),
        })
    res2 = run_bass_kernel_spmd(nc2, in_maps2, core_ids=list(range(C)),
                                trace=trace)
    if res2.exec_time_ns is not None:
        LAST_EXEC_NS.append(res2.exec_time_ns)
    out = np.sum([res2.results[k]["part"] for k in range(C)], axis=0)
    out = out + b2[None, :]
    return out.astype(np.float32)
